# revision 3
# baseline (speedup 1.0000x reference)
"""HTSK fuzzy-system kernel for Trainium2 (Bass/Tile), 8-core data-parallel.

Math (per batch row b):
  S     = H/sigma^2 + EPS                          (D,R)
  m     = mean_d(-(X_bd - C_dr)^2 * S_dr)          (B,R)
        = X^2 @ (-S/D) + X @ (2*S*C/D) + K2        (matmul expansion)
  e     = exp(m) / sum_r exp(m)                    (m <= 0 always: no max sub)
  out   = sum_r e_br * G_bro  +  e @ (W2 + 1 b^T)
  G     = X @ Wt,  Wt[d, o*R+r] = W1[r*D+d, o]     (B, O*R)  o-major

v2 schedule (vs v1):
  - X^T and (X^2)^T are prepared on the HOST (bf16) and shipped in one
    packed XIN tensor; no device transposes / squares.
  - ALL input DMAs ride the gpsimd SWDGE queues in consumption order
    (XIN, K2, then Wt quarter-by-quarter, q0/q1 split in half) - per-queue
    FIFO makes the small tensors win the HBM race without a gate, and
    SWDGE has ~4us lower first-byte latency than the sync/HWDGE path.
  - G is computed chunk-outer (Wt quarter q), tile-pair-inner so the PE
    consumes each Wt chunk for 2..4 tiles as it lands instead of
    streaming all 4MB for tile 0 first.
  - PSUM: prologue pools (warm/m/eT/out2) close before the G phase so
    ps_g can take 2 x 2048-col fp32 chunks (all 8 banks); Scalar evicts
    2048 cols per ACTIVATE (fewer instruction overheads).
  - Reduction per (tile, o-half): DVE mul (2x) + tree L1, Pool (GpSimd)
    L2+L3, DVE strided reduce_sum for the last 16 r's + out2 add. This
    splits the elementwise wall across DVE/Pool/Scalar.

Sharding: batch B=4096 split 512 rows per core; weights replicated.
"""
import sys
import types
from contextlib import ExitStack

import numpy as np

sys.path.insert(0, "/opt/trn_rl_repo")

# NTFF profile-hook registry: trn_boot sets it at jax init, concourse
# bass_utils reads it when trace=True. The container's antenv package lacks
# this submodule, so provide it before anything imports jax/concourse.
if "antenv.axon_hooks" not in sys.modules:
    _ah = types.ModuleType("antenv.axon_hooks")
    _ah._hook = None

    def _set_hook(hook):
        _ah._hook = hook

    def _get_hook():
        return _ah._hook

    _ah.set_axon_ntff_profile_hook = _set_hook
    _ah.get_axon_ntff_profile_hook = _get_hook
    sys.modules["antenv.axon_hooks"] = _ah

import ml_dtypes  # noqa: E402
import concourse.bass as bass  # noqa: E402
import concourse.bacc as bacc  # noqa: E402
import concourse.tile as tile  # noqa: E402
from concourse import mybir  # noqa: E402
from concourse import bass_utils  # noqa: E402
from concourse.masks import make_identity  # noqa: E402

H = 0.5
EPS = 1e-8
B, D, R, O = 4096, 256, 128, 64
NCORES = 8
BL = B // NCORES          # 512 batch rows per core
NT = BL // 128            # 4 partition tiles per core
RO = R * O                # 8192
HO = O // 2               # 32 o's per half
F32 = mybir.dt.float32
BF16 = mybir.dt.bfloat16

# XIN packed layout (bf16): [XT2 | XT | A | Bm | W2p]
XT2_OFF = 0
XT_OFF = NT * D                   # 1024
A_OFF = 2 * NT * D                # 2048
BM_OFF = A_OFF + 2 * R            # 2304
W2_OFF = BM_OFF + 2 * R           # 2560
XIN_W = W2_OFF + O                # 2624

_CACHE = {}


def _build():
    nc = bacc.Bacc("TRN2", target_bir_lowering=False, debug=False)
    XIN = nc.dram_tensor("XIN", [128, XIN_W], BF16, kind="ExternalInput")
    K2 = nc.dram_tensor("K2", [1, R], F32, kind="ExternalInput")
    Wt = nc.dram_tensor("Wt", [D, RO], BF16, kind="ExternalInput")
    out = nc.dram_tensor("out", [BL, O], F32, kind="ExternalOutput")

    with tile.TileContext(nc) as tc, ExitStack() as ctx:
        consts = ctx.enter_context(tc.tile_pool(name="consts", bufs=1))
        tlp = ctx.enter_context(tc.tile_pool(name="tlp", bufs=4))
        work = ctx.enter_context(tc.tile_pool(name="work", bufs=2))
        gmp = ctx.enter_context(tc.tile_pool(name="gm", bufs=1))
        treep = ctx.enter_context(tc.tile_pool(name="tree", bufs=2))

        # ---- identity (gpsimd) BEFORE the DMA pushes so it doesn't queue
        # behind 9us of push instructions ----
        identB = consts.tile([128, 128], BF16, tag="idb")
        make_identity(nc, identB)

        # ---- all input DMAs on the gpsimd SWDGE queues, consumption
        # order; per-queue FIFO = small tensors win the HBM race ----
        xin_sb = consts.tile([128, XIN_W], BF16, tag="xin")
        nc.gpsimd.dma_start(out=xin_sb[:, :], in_=XIN[:, :])
        k2_sb = consts.tile([1, R], F32, tag="k2")
        nc.gpsimd.dma_start(out=k2_sb[:, :], in_=K2[:, :])
        # Wt tiles: q0/q1 split into 1024-col halves for earlier first
        # chunks, q2/q3 as full 2048-col tiles. Arrival order below ==
        # PE consumption order.
        wt_small = {}   # (c, q, half) -> tile [128,1024]
        wt_big = {}     # (c, q) -> tile [128,2048]
        for q in (0, 1):
            for hf in (0, 1):
                for c in (0, 1):
                    t_ = consts.tile([128, 1024], BF16, tag=f"w{c}{q}{hf}")
                    nc.gpsimd.dma_start(
                        out=t_[:, :],
                        in_=Wt[c * 128:(c + 1) * 128,
                               q * 2048 + hf * 1024: q * 2048 + (hf + 1) * 1024],
                    )
                    wt_small[(c, q, hf)] = t_
        for q in (2, 3):
            for c in (0, 1):
                t_ = consts.tile([128, 2048], BF16, tag=f"w{c}{q}")
                nc.gpsimd.dma_start(
                    out=t_[:, :], in_=Wt[c * 128:(c + 1) * 128, q * 2048:(q + 1) * 2048]
                )
                wt_big[(c, q)] = t_

        def wt_rhs(c, q, off, n):
            """SBUF AP for Wt[c-chunk] columns [q*2048+off, +n)."""
            if q < 2:
                hf, o2 = divmod(off, 1024)
                return wt_small[(c, q, hf)][:, o2:o2 + n]
            return wt_big[(c, q)][:, off:off + n]

        ones_sb = consts.tile([1, 128], F32, tag="ones")
        nc.vector.memset(ones_sb, 1.0)
        warm_rhs = consts.tile([128, 512], BF16, tag="wrm")
        nc.vector.memset(warm_rhs, 1.0)
        # scalar-engine exp-table preload during the DMA window
        tbl_in = consts.tile([128, 4], F32, tag="tbi")
        nc.vector.memset(tbl_in, 0.0)
        tbl_out = consts.tile([128, 4], BF16, tag="tbo")
        nc.scalar.activation(tbl_out, tbl_in, mybir.ActivationFunctionType.Exp)

        xt2 = [[xin_sb[:, XT2_OFF + (t * 2 + c) * 128: XT2_OFF + (t * 2 + c + 1) * 128]
                for c in range(2)] for t in range(NT)]
        xt = [[xin_sb[:, XT_OFF + (t * 2 + c) * 128: XT_OFF + (t * 2 + c + 1) * 128]
               for c in range(2)] for t in range(NT)]
        a_sb = xin_sb[:, A_OFF:A_OFF + 2 * R].rearrange("p (c r) -> p c r", c=2)
        bm_sb = xin_sb[:, BM_OFF:BM_OFF + 2 * R].rearrange("p (c r) -> p c r", c=2)
        w2p_sb = xin_sb[:, W2_OFF:W2_OFF + O]

        e_ns, out2s, osbs = {}, {}, {}

        # ---- prologue phase: its PSUM pools close before ps_g opens so
        # the G phase gets all 8 banks ----
        with tc.tile_pool(name="ps_w", bufs=1, space="PSUM") as ps_w, \
             tc.tile_pool(name="ps_m", bufs=2, space="PSUM") as ps_m, \
             tc.tile_pool(name="ps_e", bufs=2, space="PSUM") as ps_e, \
             tc.tile_pool(name="ps_o", bufs=2, space="PSUM") as ps_o:
            # PE HAM warm-up: identity matmuls with no DMA deps keep the PE
            # busy through the DMA window so it ramps to 2.4 GHz early
            wm_ps = ps_w.tile([128, 512], F32, tag="wm")
            for _ in range(8):
                nc.tensor.matmul(wm_ps[:, :], lhsT=identB, rhs=warm_rhs,
                                 start=True, stop=True)

            for t in range(NT):
                m_ps = ps_m.tile([128, R], F32, tag="m", name=f"m{t}")
                nc.tensor.matmul(m_ps, lhsT=xt2[t][0], rhs=a_sb[:, 0, :],
                                 start=True, stop=False)
                nc.tensor.matmul(m_ps, lhsT=xt2[t][1], rhs=a_sb[:, 1, :],
                                 start=False, stop=False)
                nc.tensor.matmul(m_ps, lhsT=xt[t][0], rhs=bm_sb[:, 0, :],
                                 start=False, stop=False)
                nc.tensor.matmul(m_ps, lhsT=xt[t][1], rhs=bm_sb[:, 1, :],
                                 start=False, stop=False)
                nc.tensor.matmul(m_ps, lhsT=ones_sb, rhs=k2_sb,
                                 start=False, stop=True)

                # m <= 0 always (S > 0), so exp never overflows: skip max
                e_bf = work.tile([128, R], BF16, tag="e", name=f"e{t}")
                s_ = work.tile([128, 1], F32, tag="s", name=f"s{t}")
                nc.scalar.activation(e_bf, m_ps, mybir.ActivationFunctionType.Exp,
                                     accum_out=s_)
                rs = work.tile([128, 1], F32, tag="rs", name=f"rs{t}")
                nc.vector.reciprocal(rs, s_)
                e_n = tlp.tile([128, R], BF16, tag="en", name=f"en{t}")
                nc.vector.tensor_scalar_mul(e_n, e_bf, rs)

                eT_ps = ps_e.tile([128, 128], BF16, tag="eT", name=f"eT{t}")
                nc.tensor.transpose(eT_ps, e_n, identB)
                eT_sb = work.tile([128, 128], BF16, tag="eTsb", name=f"eTsb{t}")
                nc.vector.tensor_copy(eT_sb, eT_ps)
                out2_ps = ps_o.tile([128, O], F32, tag="o2", name=f"o2{t}")
                nc.tensor.matmul(out2_ps, lhsT=eT_sb, rhs=w2p_sb,
                                 start=True, stop=True)
                out2_sb = tlp.tile([128, O], F32, tag="o2sb", name=f"o2sb{t}")
                nc.vector.tensor_copy(out2_sb, out2_ps)

                e_ns[t], out2s[t] = e_n, out2_sb
                osbs[t] = tlp.tile([128, O], F32, tag="osb", name=f"osb{t}")
                gm = gmp.tile([128, RO], BF16, tag=f"gm{t}", name=f"gm{t}")
                osbs[t] = (osbs[t], gm)

        # ---- G phase ----
        with tc.tile_pool(name="ps_g", bufs=2, space="PSUM") as ps_g:

            def g_chunk(q, t):
                """G columns [q*2048, (q+1)*2048) for tile t: 4 psum groups
                of 512 cols, 2 c-accumulated matmuls each; Scalar evicts the
                whole 2048-col chunk to gm in one ACTIVATE."""
                gm = osbs[t][1]
                gt = ps_g.tile([128, 2048], F32, tag="g", name=f"g{q}_{t}")
                for grp in range(4):
                    for c in range(2):
                        nc.tensor.matmul(
                            gt[:, grp * 512:(grp + 1) * 512],
                            lhsT=xt[t][c],
                            rhs=wt_rhs(c, q, grp * 512, 512),
                            start=(c == 0), stop=(c == 1),
                        )
                nc.scalar.copy(gm[:, q * 2048:(q + 1) * 2048], gt)

            def unit(t, h):
                """e-multiply + r-reduction for tile t, o-half h (o in
                [32h, 32h+32)): DVE mul + L1, Pool L2 + L3, DVE tail
                reduce + out2 add. DMAs the tile's output after h==1."""
                e_n = e_ns[t]
                osb, gm = osbs[t]
                gv = gm[:, h * 4096:(h + 1) * 4096].rearrange(
                    "p (o r) -> p o r", o=HO)
                ebc = e_n.rearrange("p r -> p () r").broadcast_to((128, HO, R))
                nc.vector.tensor_mul(gv, gv, ebc)

                l1 = treep.tile([128, HO * 64], BF16, tag=f"l1{h}",
                                name=f"l1_{t}_{h}")
                l1v = l1.rearrange("p (o r) -> p o r", o=HO)
                nc.vector.tensor_add(l1v, gv[:, :, 0:64], gv[:, :, 64:128])
                l2 = treep.tile([128, HO * 32], BF16, tag=f"l2{h}",
                                name=f"l2_{t}_{h}")
                l2v = l2.rearrange("p (o r) -> p o r", o=HO)
                nc.gpsimd.tensor_add(l2v, l1v[:, :, 0:32], l1v[:, :, 32:64])
                l3 = treep.tile([128, HO * 16], BF16, tag=f"l3{h}",
                                name=f"l3_{t}_{h}")
                l3v = l3.rearrange("p (o r) -> p o r", o=HO)
                nc.gpsimd.tensor_add(l3v, l2v[:, :, 0:16], l2v[:, :, 16:32])

                red = work.tile([128, HO], F32, tag=f"red{h}",
                                name=f"red_{t}_{h}")
                nc.vector.reduce_sum(red, l3v, axis=mybir.AxisListType.X)
                hsl = slice(h * HO, (h + 1) * HO)
                nc.vector.tensor_add(osb[:, hsl], red, out2s[t][:, hsl])
                if h == 1:
                    nc.sync.dma_start(out=out[t * 128:(t + 1) * 128, :], in_=osb)

            # chunk-outer / tile-pair-inner order matched to Wt arrival;
            # each unit is emitted as soon as both its q-chunks are evicted
            g_chunk(0, 0); g_chunk(0, 1)
            g_chunk(1, 0); g_chunk(1, 1)
            unit(0, 0)
            g_chunk(0, 2); g_chunk(0, 3)
            unit(1, 0)
            g_chunk(1, 2); g_chunk(1, 3)
            unit(2, 0); unit(3, 0)
            g_chunk(2, 0); g_chunk(2, 1)
            g_chunk(3, 0); g_chunk(3, 1)
            unit(0, 1)
            g_chunk(2, 2); g_chunk(2, 3)
            unit(1, 1)
            g_chunk(3, 2); g_chunk(3, 3)
            unit(2, 1); unit(3, 1)

    nc.finalize()
    return nc


def _get_nc():
    if "nc" not in _CACHE:
        _CACHE["nc"] = _build()
    return _CACHE["nc"]


def _host_prep(centers, sigmas, W, b):
    c64 = centers.astype(np.float64)
    S = (H / sigmas.astype(np.float64) ** 2) + EPS          # (D,R)
    A = (-S / D).astype(ml_dtypes.bfloat16)                  # X^2 coeff
    Bm = (2.0 * S * c64 / D).astype(ml_dtypes.bfloat16)      # X coeff
    K2 = (-(S * c64 * c64).sum(axis=0, keepdims=True) / D).astype(np.float32)
    W1 = W[: D * R].reshape(R, D, O)
    # o-major: Wt[d, o*R + r] = W1[r, d, o]
    Wt = np.ascontiguousarray(W1.transpose(1, 2, 0).reshape(D, RO)).astype(
        ml_dtypes.bfloat16
    )
    W2p = (W[D * R:].astype(np.float64) + b[None, :].astype(np.float64)).astype(
        ml_dtypes.bfloat16
    )
    # CB = [A (c-major) | Bm (c-major) | W2p], all bf16 [128, 576]
    CB = np.concatenate([
        A.reshape(2, 128, R).transpose(1, 0, 2).reshape(128, 2 * R),
        Bm.reshape(2, 128, R).transpose(1, 0, 2).reshape(128, 2 * R),
        W2p,
    ], axis=1)
    return np.ascontiguousarray(CB), K2, Wt


def _xin_for_core(Xc, CB):
    """[XT2 | XT | CB] bf16, XT[p, (t*2+c)*128+q] = Xc[t*128+q, c*128+p]."""
    xr = Xc.reshape(NT, 128, 2, 128)
    XT = xr.transpose(3, 0, 2, 1).reshape(128, NT * D)
    XT2 = (xr * xr).transpose(3, 0, 2, 1).reshape(128, NT * D)
    return np.ascontiguousarray(np.concatenate(
        [XT2.astype(ml_dtypes.bfloat16), XT.astype(ml_dtypes.bfloat16), CB],
        axis=1))


def kernel(X, centers, sigmas, W, b):
    X = np.asarray(X, dtype=np.float32)
    centers = np.asarray(centers, dtype=np.float32)
    sigmas = np.asarray(sigmas, dtype=np.float32)
    W = np.asarray(W, dtype=np.float32)
    b = np.asarray(b, dtype=np.float32)

    CB, K2, Wt = _host_prep(centers, sigmas, W, b)
    nc = _get_nc()
    in_maps = [
        {
            "XIN": _xin_for_core(X[k * BL:(k + 1) * BL], CB),
            "K2": K2, "Wt": Wt,
        }
        for k in range(NCORES)
    ]
    res = bass_utils.run_bass_kernel_spmd(nc, in_maps, core_ids=list(range(NCORES)))
    return np.concatenate([res.results[k]["out"] for k in range(NCORES)], axis=0)


# revision 18
# speedup vs baseline: 1.1800x; 1.1800x over previous
"""HTSK fuzzy-system kernel for Trainium2 (Bass/Tile), 8-core data-parallel.

Math (per batch row b):
  S     = H/sigma^2 + EPS                          (D,R)
  m     = mean_d(-(X_bd - C_dr)^2 * S_dr)          (B,R)
        = X^2 @ (-S/D) + X @ (2*S*C/D) + K2        (matmul expansion)
  e     = exp(m) / sum_r exp(m)                    (m <= 0 always: no max sub)
  out   = sum_r e_br * G_bro  +  e @ (W2 + 1 b^T)
  G     = X @ Wt,  Wt[d, o*R+r] = W1[r*D+d, o]     (B, O*R)  o-major

v2 schedule (vs v1):
  - X^T and (X^2)^T are prepared on the HOST (bf16) and shipped in one
    packed XIN tensor; no device transposes / squares.
  - ALL input DMAs ride the gpsimd SWDGE queues in consumption order
    (XIN, K2, then Wt quarter-by-quarter, q0/q1 split in half) - per-queue
    FIFO makes the small tensors win the HBM race without a gate, and
    SWDGE has ~4us lower first-byte latency than the sync/HWDGE path.
  - G is computed chunk-outer (Wt quarter q), tile-pair-inner so the PE
    consumes each Wt chunk for 2..4 tiles as it lands instead of
    streaming all 4MB for tile 0 first.
  - PSUM: prologue pools (warm/m/eT/out2) close before the G phase so
    ps_g can take 2 x 2048-col fp32 chunks (all 8 banks); Scalar evicts
    2048 cols per ACTIVATE (fewer instruction overheads).
  - Reduction per (tile, o-half): DVE mul (2x) + tree L1, Pool (GpSimd)
    L2+L3, DVE strided reduce_sum for the last 16 r's + out2 add. This
    splits the elementwise wall across DVE/Pool/Scalar.

Sharding: batch B=4096 split 512 rows per core; weights replicated.
"""
import sys
import types
from contextlib import ExitStack

import numpy as np

sys.path.insert(0, "/opt/trn_rl_repo")

# NTFF profile-hook registry: trn_boot sets it at jax init, concourse
# bass_utils reads it when trace=True. The container's antenv package lacks
# this submodule, so provide it before anything imports jax/concourse.
if "antenv.axon_hooks" not in sys.modules:
    _ah = types.ModuleType("antenv.axon_hooks")
    _ah._hook = None

    def _set_hook(hook):
        _ah._hook = hook

    def _get_hook():
        return _ah._hook

    _ah.set_axon_ntff_profile_hook = _set_hook
    _ah.get_axon_ntff_profile_hook = _get_hook
    sys.modules["antenv.axon_hooks"] = _ah

import ml_dtypes  # noqa: E402
import concourse.bass as bass  # noqa: E402
import concourse.bacc as bacc  # noqa: E402
import concourse.tile as tile  # noqa: E402
from concourse import mybir  # noqa: E402
from concourse import bass_utils  # noqa: E402
from concourse.masks import make_identity  # noqa: E402

H = 0.5
EPS = 1e-8
B, D, R, O = 4096, 256, 128, 64
NCORES = 8
BL = B // NCORES          # 512 batch rows per core
NT = BL // 128            # 4 partition tiles per core
RO = R * O                # 8192
HO = O // 2               # 32 o's per half
F32 = mybir.dt.float32
BF16 = mybir.dt.bfloat16

# XIN packed layout (bf16): [XT2 | XT | A | Bm | W2p]
XT2_OFF = 0
XT_OFF = NT * D                   # 1024
A_OFF = 2 * NT * D                # 2048
BM_OFF = A_OFF + 2 * R            # 2304
W2_OFF = BM_OFF + 2 * R           # 2560
XIN_W = W2_OFF + O                # 2624

_CACHE = {}


def _build():
    nc = bacc.Bacc("TRN2", target_bir_lowering=False, debug=False)
    XIN = nc.dram_tensor("XIN", [128, XIN_W], BF16, kind="ExternalInput")
    K2 = nc.dram_tensor("K2", [1, R], F32, kind="ExternalInput")
    Wt = nc.dram_tensor("Wt", [D, RO], BF16, kind="ExternalInput")
    out = nc.dram_tensor("out", [BL, O], F32, kind="ExternalOutput")

    with tile.TileContext(nc) as tc, ExitStack() as ctx:
        consts = ctx.enter_context(tc.tile_pool(name="consts", bufs=1))
        tlp = ctx.enter_context(tc.tile_pool(name="tlp", bufs=4))
        work = ctx.enter_context(tc.tile_pool(name="work", bufs=2))
        gmp = ctx.enter_context(tc.tile_pool(name="gm", bufs=4))
        treep = ctx.enter_context(tc.tile_pool(name="tree", bufs=2))

        # ---- dummy SWDGE transfer first: warms the Q7 descriptor path /
        # rings so the real input DMAs don't eat the cold-start latency ----
        dummy = consts.tile([1, 32], BF16, tag="dmy")
        nc.gpsimd.dma_start(out=dummy[:, :], in_=Wt[0:1, 0:32])

        # identity (gpsimd) before the big pushes (cheap, needed by ~8us)
        identB = consts.tile([128, 128], BF16, tag="idb")
        make_identity(nc, identB)

        # ---- all input DMAs on the gpsimd SWDGE queues, consumption
        # order; per-queue FIFO = small tensors win the HBM race ----
        xin_sb = consts.tile([128, XIN_W], BF16, tag="xin")
        nc.gpsimd.dma_start(out=xin_sb[:, :], in_=XIN[:, :])
        k2_sb = consts.tile([1, R], F32, tag="k2")
        nc.gpsimd.dma_start(out=k2_sb[:, :], in_=K2[:, :])
        # Wt tiles: q0 split into 1024-col halves for an earlier first
        # chunk, q1-q3 as full 2048-col tiles. Arrival order below ==
        # PE consumption order.
        wt_small = {}   # (c, 0, half) -> tile [128,1024]
        wt_big = {}     # (c, q) -> tile [128,2048]
        for hf in (0, 1):
            for c in (0, 1):
                t_ = consts.tile([128, 1024], BF16, tag=f"w{c}0{hf}")
                nc.gpsimd.dma_start(
                    out=t_[:, :],
                    in_=Wt[c * 128:(c + 1) * 128, hf * 1024:(hf + 1) * 1024],
                )
                wt_small[(c, 0, hf)] = t_
        for q in (1, 2, 3):
            for c in (0, 1):
                t_ = consts.tile([128, 2048], BF16, tag=f"w{c}{q}")
                nc.gpsimd.dma_start(
                    out=t_[:, :], in_=Wt[c * 128:(c + 1) * 128, q * 2048:(q + 1) * 2048]
                )
                wt_big[(c, q)] = t_

        def wt_rhs(c, q, off, n):
            """SBUF AP for Wt[c-chunk] columns [q*2048+off, +n)."""
            if q < 1:
                hf, o2 = divmod(off, 1024)
                return wt_small[(c, q, hf)][:, o2:o2 + n]
            return wt_big[(c, q)][:, off:off + n]

        ones_sb = consts.tile([1, 128], F32, tag="ones")
        nc.vector.memset(ones_sb, 1.0)
        warm_rhs = consts.tile([128, 512], BF16, tag="wrm")
        nc.vector.memset(warm_rhs, 1.0)
        # scalar-engine exp-table preload during the DMA window
        tbl_in = consts.tile([128, 4], F32, tag="tbi")
        nc.vector.memset(tbl_in, 0.0)
        tbl_out = consts.tile([128, 4], BF16, tag="tbo")
        nc.scalar.activation(tbl_out, tbl_in, mybir.ActivationFunctionType.Exp)

        xt2 = [[xin_sb[:, XT2_OFF + (t * 2 + c) * 128: XT2_OFF + (t * 2 + c + 1) * 128]
                for c in range(2)] for t in range(NT)]
        xt = [[xin_sb[:, XT_OFF + (t * 2 + c) * 128: XT_OFF + (t * 2 + c + 1) * 128]
               for c in range(2)] for t in range(NT)]
        a_sb = xin_sb[:, A_OFF:A_OFF + 2 * R].rearrange("p (c r) -> p c r", c=2)
        bm_sb = xin_sb[:, BM_OFF:BM_OFF + 2 * R].rearrange("p (c r) -> p c r", c=2)
        w2p_sb = xin_sb[:, W2_OFF:W2_OFF + O]

        e_ns, out2s, osbs = {}, {}, {}

        ps_m = ctx.enter_context(tc.tile_pool(name="ps_m", bufs=1, space="PSUM"))
        ps_s = ctx.enter_context(tc.tile_pool(name="ps_s", bufs=1, space="PSUM"))
        ps_g = ctx.enter_context(tc.tile_pool(name="ps_g", bufs=3, space="PSUM"))

        # PE HAM warm-up: identity matmuls with no DMA deps keep the PE
        # busy through the DMA window so it ramps to 2.4 GHz early.
        # Uses a ps_g slot (released before the first real G chunk).
        wm_ps = ps_g.tile([128, 1024], F32, tag="g", name="warm")
        for _ in range(8):
            nc.tensor.matmul(wm_ps[:, 0:512], lhsT=identB, rhs=warm_rhs,
                             start=True, stop=True)

        if True:
            for t in range(NT):
                m_ps = ps_m.tile([128, R], F32, tag="m", name=f"m{t}")
                nc.tensor.matmul(m_ps, lhsT=xt2[t][0], rhs=a_sb[:, 0, :],
                                 start=True, stop=False)
                nc.tensor.matmul(m_ps, lhsT=xt2[t][1], rhs=a_sb[:, 1, :],
                                 start=False, stop=False)
                nc.tensor.matmul(m_ps, lhsT=xt[t][0], rhs=bm_sb[:, 0, :],
                                 start=False, stop=False)
                nc.tensor.matmul(m_ps, lhsT=xt[t][1], rhs=bm_sb[:, 1, :],
                                 start=False, stop=False)
                nc.tensor.matmul(m_ps, lhsT=ones_sb, rhs=k2_sb,
                                 start=False, stop=True)

                # m <= 0 always (S > 0), so exp never overflows: skip max
                e_bf = work.tile([128, R], BF16, tag="e", name=f"e{t}")
                s_ = work.tile([128, 1], F32, tag="s", name=f"s{t}")
                nc.scalar.activation(e_bf, m_ps, mybir.ActivationFunctionType.Exp,
                                     accum_out=s_)
                rs = work.tile([128, 1], F32, tag="rs", name=f"rs{t}")
                nc.vector.reciprocal(rs, s_)
                e_n = tlp.tile([128, R], BF16, tag="en", name=f"en{t}")
                nc.vector.tensor_scalar_mul(e_n, e_bf, rs)

                eT_ps = ps_s.tile([128, 128], BF16, tag="s", name=f"eT{t}")
                nc.tensor.transpose(eT_ps, e_n, identB)
                eT_sb = work.tile([128, 128], BF16, tag="eTsb", name=f"eTsb{t}")
                nc.vector.tensor_copy(eT_sb, eT_ps)
                out2_ps = ps_s.tile([128, O], F32, tag="s", name=f"o2{t}")
                nc.tensor.matmul(out2_ps, lhsT=eT_sb, rhs=w2p_sb,
                                 start=True, stop=True)
                out2_sb = tlp.tile([128, O], F32, tag="o2sb", name=f"o2sb{t}")
                nc.vector.tensor_copy(out2_sb, out2_ps)

                e_ns[t], out2s[t] = e_n, out2_sb
                osbs[t] = tlp.tile([128, O], F32, tag="osb", name=f"osb{t}")

        # ---- G phase ----
        # m2[t] holds e*G for a whole tile re-blocked into 4 contiguous
        # r-quarter planes: col = q*2048 + o*32 + r'  (o global 0..63).
        # One 1MB CCE accumulate-DMA folds quarters {2,3} onto {0,1}
        # (r 128->64); DVE folds the rest.
        m2p = ctx.enter_context(tc.tile_pool(name="m2", bufs=4))
        gmhs = {}

        m2s = {}

        def _m2(t):
            if t not in m2s:
                m2s[t] = m2p.tile([128, RO], BF16, tag="m2", name=f"m2_{t}")
            return m2s[t]

        def _m2v(t):
            # (p, quarter, o_global, r') view: col = q*2048 + o*32 + r'
            return _m2(t).rearrange("p (q o r) -> p q o r", q=4, o=O)

        def _eq(t, on):
            return e_ns[t].rearrange("p (q r) -> p q () r", q=4).broadcast_to(
                (128, 4, on, 32))

        def g_chunk(j, t):
            """G columns [j*1024, (j+1)*1024) for tile t: 2 psum groups of
            512 cols, 2 c-accumulated matmuls each. Chunk j==4 is evicted
            by a DVE fused evict+e-multiply straight into m2 (1x PSUM
            read); the rest go through Scalar into the half-buffer gmh."""
            gt = ps_g.tile([128, 1024], F32, tag="g", name=f"g{j}_{t}")
            q, off = divmod(j * 1024, 2048)
            for grp in range(2):
                for c in range(2):
                    nc.tensor.matmul(
                        gt[:, grp * 512:(grp + 1) * 512],
                        lhsT=xt[t][c],
                        rhs=wt_rhs(c, q, off + grp * 512, 512),
                        start=(c == 0), stop=(c == 1),
                    )
            gmh = gmhs[(t, j // 4)]
            nc.scalar.copy(gmh[:, (j % 4) * 1024:(j % 4 + 1) * 1024], gt)

        def half_mul(t, h):
            """DVE e-multiply for tile t's o-half h, reading the gmh evict
            buffer and writing the re-blocked quarter planes of m2[t]."""
            gmh = gmhs.pop((t, h))
            gq = gmh.rearrange("p (o q r) -> p q o r", o=32, q=4)
            mq = _m2v(t)[:, :, 32 * h:32 * (h + 1), :]
            nc.vector.tensor_mul(mq, gq, _eq(t, 32))

        def tile_tail(t):
            """r 128->64 on the CCE adders (one 1MB contiguous accumulate
            DMA), then DVE: r 64->32 flat add, 32->16->8 strided adds, 8->1
            reduce, + out2, and the tile's output DMA."""
            m2 = _m2(t)
            nc.gpsimd.dma_start(out=m2[:, 0:2048], in_=m2[:, 4096:6144],
                                accum_op=mybir.AluOpType.add)
            nc.gpsimd.dma_start(out=m2[:, 2048:4096], in_=m2[:, 6144:8192],
                                accum_op=mybir.AluOpType.add)
            nc.vector.tensor_add(m2[:, 0:2048], m2[:, 0:2048], m2[:, 2048:4096])
            mv = m2[:, 0:2048].rearrange("p (o r) -> p o r", o=O)
            l3 = treep.tile([128, O * 16], BF16, tag="l3", name=f"l3_{t}")
            l3v = l3.rearrange("p (o r) -> p o r", o=O)
            nc.vector.tensor_add(l3v, mv[:, :, 0:16], mv[:, :, 16:32])
            l4 = treep.tile([128, O * 8], BF16, tag="l4", name=f"l4_{t}")
            l4v = l4.rearrange("p (o r) -> p o r", o=O)
            nc.vector.tensor_add(l4v, l3v[:, :, 0:8], l3v[:, :, 8:16])
            red = work.tile([128, O], F32, tag="red", name=f"red_{t}")
            nc.vector.reduce_sum(red, l4v, axis=mybir.AxisListType.X)
            osb = osbs[t]
            nc.vector.tensor_add(osb, red, out2s[t])
            nc.sync.dma_start(out=out[t * 128:(t + 1) * 128, :], in_=osb)

        def quad(j0, ta, tb):
            for j in (j0, j0 + 1):
                for t in (ta, tb):
                    if (t, j // 4) not in gmhs:
                        gmhs[(t, j // 4)] = gmp.tile(
                            [128, 4096], BF16, tag="gmh", name=f"gmh_{t}_{j // 4}")
                    g_chunk(j, t)

        # chunk-outer / tile-pair-inner order matched to Wt arrival
        quad(0, 0, 1)
        quad(2, 0, 1)
        half_mul(0, 0); half_mul(1, 0)
        quad(0, 2, 3)
        quad(2, 2, 3)
        half_mul(2, 0); half_mul(3, 0)
        quad(4, 0, 1)
        quad(6, 0, 1)
        half_mul(0, 1); half_mul(1, 1)
        tile_tail(0); tile_tail(1)
        quad(4, 2, 3)
        quad(6, 2, 3)
        half_mul(2, 1); half_mul(3, 1)
        tile_tail(2); tile_tail(3)

    nc.finalize()
    return nc


def _get_nc():
    if "nc" not in _CACHE:
        _CACHE["nc"] = _build()
    return _CACHE["nc"]


def _host_prep(centers, sigmas, W, b):
    c64 = centers.astype(np.float64)
    S = (H / sigmas.astype(np.float64) ** 2) + EPS          # (D,R)
    A = (-S / D).astype(ml_dtypes.bfloat16)                  # X^2 coeff
    Bm = (2.0 * S * c64 / D).astype(ml_dtypes.bfloat16)      # X coeff
    K2 = (-(S * c64 * c64).sum(axis=0, keepdims=True) / D).astype(np.float32)
    W1 = W[: D * R].reshape(R, D, O)
    # o-major: Wt[d, o*R + r] = W1[r, d, o]
    Wt = np.ascontiguousarray(W1.transpose(1, 2, 0).reshape(D, RO)).astype(
        ml_dtypes.bfloat16
    )
    W2p = (W[D * R:].astype(np.float64) + b[None, :].astype(np.float64)).astype(
        ml_dtypes.bfloat16
    )
    # CB = [A (c-major) | Bm (c-major) | W2p], all bf16 [128, 576]
    CB = np.concatenate([
        A.reshape(2, 128, R).transpose(1, 0, 2).reshape(128, 2 * R),
        Bm.reshape(2, 128, R).transpose(1, 0, 2).reshape(128, 2 * R),
        W2p,
    ], axis=1)
    return np.ascontiguousarray(CB), K2, Wt


def _xin_for_core(Xc, CB):
    """[XT2 | XT | CB] bf16, XT[p, (t*2+c)*128+q] = Xc[t*128+q, c*128+p]."""
    xr = Xc.reshape(NT, 128, 2, 128)
    XT = xr.transpose(3, 0, 2, 1).reshape(128, NT * D)
    XT2 = (xr * xr).transpose(3, 0, 2, 1).reshape(128, NT * D)
    return np.ascontiguousarray(np.concatenate(
        [XT2.astype(ml_dtypes.bfloat16), XT.astype(ml_dtypes.bfloat16), CB],
        axis=1))


def kernel(X, centers, sigmas, W, b):
    X = np.asarray(X, dtype=np.float32)
    centers = np.asarray(centers, dtype=np.float32)
    sigmas = np.asarray(sigmas, dtype=np.float32)
    W = np.asarray(W, dtype=np.float32)
    b = np.asarray(b, dtype=np.float32)

    CB, K2, Wt = _host_prep(centers, sigmas, W, b)
    nc = _get_nc()
    in_maps = [
        {
            "XIN": _xin_for_core(X[k * BL:(k + 1) * BL], CB),
            "K2": K2, "Wt": Wt,
        }
        for k in range(NCORES)
    ]
    res = bass_utils.run_bass_kernel_spmd(nc, in_maps, core_ids=list(range(NCORES)))
    return np.concatenate([res.results[k]["out"] for k in range(NCORES)], axis=0)
